# revision 12
# baseline (speedup 1.0000x reference)
"""Trainium2 Bass kernel for nn_Network_28054726377822 (LSTM, B=64 T=1024 D=512 U=512 OUT=4).

Strategy:
  - Data-parallel: batch (64) sharded 8 ways across cores (8 samples/core).
  - Phase 1 (per core): xz = tx @ kernel + bias as a bf16 GEMM (fp32 accumulate),
    written to DRAM scratch with host-permuted columns.
  - Phase 2: 1024-step LSTM recurrence. Per step:
      * xz_t DMA'd into a "sparse packed" SBUF tile [128p, 512f] where
        partition = 32*hb + b (hb = hidden-block of 128 units, b = sample),
        free = gate*128 + jl. Injected into PSUM via an identity matmul
        (start=True), which also solves the has_written accumulate gotcha.
      * z += h @ R via 16 matmuls (4 hidden-groups x 4 K-tiles) streaming the
        column-permuted recurrent kernel R_perm (bf16, resident in SBUF).
      * Gates on ScalarE (tanh/sigmoid on short free dims), c/h updates on
        VectorE, h transposed back to hT layout via one PE transpose + 4 copies.
  - Phase 3 (host): out = softmax(h_last @ fc_w + fc_b) in fp32 numpy.

Self-contained: hardcodes all shapes; sharding/gather done here in numpy.
"""

import numpy as np
import ml_dtypes

B, T, D, U, OUT = 64, 1024, 512, 512, 4
NCORES = 8
BL = B // NCORES          # 8 samples per core
HB = 4                    # hidden blocks of 128
JL = U // HB              # 128
G4 = 4 * U                # 2048

BF16 = ml_dtypes.bfloat16


def _perm_cols():
    """col-perm: new col hb*512 + g*128 + jl  <-  old col g*512 + hb*128 + jl."""
    idx = np.empty(G4, dtype=np.int64)
    for hb in range(HB):
        for g in range(4):
            for jl in range(JL):
                idx[hb * 512 + g * 128 + jl] = g * 512 + hb * 128 + jl
    return idx


_PERM = _perm_cols()


def _build_bass():
    import concourse.mybir as mybir
    import concourse.tile as tile
    from concourse import bacc
    from concourse.masks import make_identity

    dt = mybir.dt
    nc = bacc.Bacc("TRN2", target_bir_lowering=False, num_devices=NCORES)

    # ---- I/O ----
    txT_d = nc.dram_tensor("txT", [D, BL * T], dt.bfloat16, kind="ExternalInput").ap()
    kern_d = nc.dram_tensor("kern_perm", [D, G4], dt.bfloat16, kind="ExternalInput").ap()
    r_d = nc.dram_tensor("r_perm", [D, G4], dt.bfloat16, kind="ExternalInput").ap()
    bias_d = nc.dram_tensor("bias_perm", [1, G4], dt.bfloat16, kind="ExternalInput").ap()
    hT_out_d = nc.dram_tensor("hT_out", [JL, HB, BL], dt.float32, kind="ExternalOutput").ap()
    # DRAM scratch for xz, layout [t, hb, b, f(g*128+jl)]
    xz_d = nc.dram_tensor("xz_scratch", [T, HB, BL, 512], dt.bfloat16, kind="Internal").ap()

    with tile.TileContext(nc) as tc:
        # ---------- constants ----------
        const = tc.tile_pool(name="const", bufs=1)
        with const as cpool:
            kern_sb = cpool.tile([128, 4, G4], dt.bfloat16, tag="kern")
            r_sb = cpool.tile([128, 4, G4], dt.bfloat16, tag="rsb")
            for k in range(4):
                nc.gpsimd.dma_start(out=kern_sb[:, k, :], in_=kern_d[128 * k : 128 * k + 128, :])
                nc.gpsimd.dma_start(out=r_sb[:, k, :], in_=r_d[128 * k : 128 * k + 128, :])
            bias_sb = cpool.tile([1, G4], dt.bfloat16, tag="bias")
            nc.gpsimd.dma_start(out=bias_sb, in_=bias_d)
            ones_sb = cpool.tile([1, 128], dt.bfloat16, tag="ones")
            nc.vector.memset(ones_sb, 1.0)
            ident = cpool.tile([128, 128], dt.bfloat16, tag="ident")
            make_identity(nc, ident)

            # persistent recurrence state (double-buffered by hand)
            hT_state = cpool.tile([128, 2, HB, BL], dt.bfloat16, tag="hT")
            nc.vector.memset(hT_state, 0.0)
            c_state = cpool.tile([128, 2, JL], dt.float32, tag="c")
            nc.vector.memset(c_state, 0.0)
            # xz staging, manually double buffered; memset once so the unused
            # partitions (24 of every 32) hold finite values.
            xz_stage = cpool.tile([128, 2, 512], dt.bfloat16, tag="xzs")
            nc.vector.memset(xz_stage, 0.0)

            # ---------- phase 1: xz = tx @ kernel + bias ----------
            with (
                tc.tile_pool(name="p1ps", bufs=2, space="PSUM") as p1ps,
                tc.tile_pool(name="p1sb", bufs=3) as p1sb,
            ):
                nblocks = BL * T // 128
                for rb in range(nblocks):
                    b_i, t0 = rb // (T // 128), (rb % (T // 128)) * 128
                    lhs = p1sb.tile([128, 4, 128], dt.bfloat16, tag="lhs")
                    for k in range(4):
                        nc.gpsimd.dma_start(
                            out=lhs[:, k, :],
                            in_=txT_d[128 * k : 128 * k + 128, rb * 128 : rb * 128 + 128],
                        )
                    ps = p1ps.tile([128, 4, 512], dt.float32, tag="p1z")
                    for g in range(4):
                        for k in range(4):
                            nc.tensor.matmul(
                                ps[:, g, :],
                                lhsT=lhs[:, k, :],
                                rhs=kern_sb[:, k, g * 512 : g * 512 + 512],
                                start=(k == 0),
                                stop=False,
                            )
                        nc.tensor.matmul(
                            ps[:, g, :],
                            lhsT=ones_sb,
                            rhs=bias_sb[:, g * 512 : g * 512 + 512],
                            start=False,
                            stop=True,
                        )
                    xzo = p1sb.tile([128, 4, 512], dt.bfloat16, tag="xzo")
                    nc.scalar.copy(out=xzo, in_=ps)
                    # dest [t, hb, b_i, f] for t in [t0, t0+128)
                    for hb in range(HB):
                        nc.gpsimd.dma_start(
                            out=xz_d[t0 : t0 + 128, hb, b_i, :],
                            in_=xzo[:, hb, :],
                        )

            tc.strict_bb_all_engine_barrier()

            # ---------- phase 2: recurrence ----------
            with tc.tile_pool(name="p2ps", bufs=2, space="PSUM") as p2ps, \
                 tc.tile_pool(name="p2t", bufs=2, space="PSUM") as p2t, \
                 tc.tile_pool(name="p2sb", bufs=3) as p2sb:

                for t in range(T):
                    cur, nxt = t % 2, (t + 1) % 2
                    xz_sb = xz_stage[:, cur, :]
                    for hb in range(HB):
                        nc.gpsimd.dma_start(
                            out=xz_sb[32 * hb : 32 * hb + BL, :],
                            in_=xz_d[t, hb],
                        )
                    z_ps = p2ps.tile([128, 512], dt.float32, tag="z")
                    # inject xz (+ sets has_written on all partitions)
                    nc.tensor.matmul(z_ps, lhsT=ident, rhs=xz_sb, start=True, stop=False)
                    hT_cur = hT_state[:, cur]
                    for hb in range(HB):
                        for k in range(4):
                            nc.tensor.matmul(
                                z_ps[32 * hb : 32 * hb + BL, :],
                                lhsT=hT_cur[:, k, :],
                                rhs=r_sb[:, k, hb * 512 : hb * 512 + 512],
                                start=False,
                                stop=(hb == HB - 1 and k == 3),
                                skip_group_check=True,
                                tile_position=(0, 32 * hb),
                            )
                    # gates
                    v1 = p2sb.tile([128, 128], dt.bfloat16, tag="v1")
                    nc.scalar.activation(v1, z_ps[:, 0:128], mybir.ActivationFunctionType.Tanh)
                    v234 = p2sb.tile([128, 384], dt.bfloat16, tag="v234")
                    nc.scalar.activation(v234, z_ps[:, 128:512], mybir.ActivationFunctionType.Sigmoid)
                    m1 = p2sb.tile([128, 128], dt.bfloat16, tag="m1")
                    nc.vector.tensor_mul(m1, v1, v234[:, 0:128])
                    m2 = p2sb.tile([128, 128], dt.float32, tag="m2")
                    nc.vector.tensor_mul(m2, v234[:, 128:256], c_state[:, cur])
                    c_new = c_state[:, nxt]
                    nc.vector.tensor_add(c_new, m1, m2)
                    tc_t = p2sb.tile([128, 128], dt.bfloat16, tag="tc")
                    nc.scalar.activation(tc_t, c_new, mybir.ActivationFunctionType.Tanh)
                    h_t = p2sb.tile([128, 128], dt.bfloat16, tag="h")
                    nc.vector.tensor_mul(h_t, v234[:, 256:384], tc_t)
                    # transpose h back to hT layout
                    hTT = p2t.tile([128, 128], dt.bfloat16, tag="hTT")
                    nc.tensor.transpose(hTT, h_t, ident)
                    hT_new = hT_state[:, nxt]
                    for hb in range(HB):
                        nc.vector.tensor_copy(
                            hT_new[:, hb, :], hTT[:, 32 * hb : 32 * hb + BL]
                        )

            tc.strict_bb_all_engine_barrier()
            # write out final hT (fp32 for host convenience)
            hT_f32 = cpool.tile([128, HB, BL], dt.float32, tag="hTf")
            nc.vector.tensor_copy(hT_f32, hT_state[:, T % 2])
            nc.gpsimd.dma_start(
                out=hT_out_d.rearrange("p hb b -> p (hb b)"),
                in_=hT_f32.rearrange("p hb b -> p (hb b)"),
            )

    nc.compile()
    return nc


_NC_CACHE = None
LAST_RESULTS = None  # BassKernelResults from the most recent run (for profiling)


def kernel(tx, kernel, recurrent_kernel, bias, fc_w, fc_b):
    global _NC_CACHE, LAST_RESULTS
    from concourse.bass_utils import run_bass_kernel_spmd

    tx = np.asarray(tx, dtype=np.float32)
    kern = np.asarray(kernel, dtype=np.float32)
    R = np.asarray(recurrent_kernel, dtype=np.float32)
    bias = np.asarray(bias, dtype=np.float32)
    fc_w = np.asarray(fc_w, dtype=np.float32)
    fc_b = np.asarray(fc_b, dtype=np.float32)

    kern_perm = np.ascontiguousarray(kern[:, _PERM]).astype(BF16)
    r_perm = np.ascontiguousarray(R[:, _PERM]).astype(BF16)
    bias_perm = np.ascontiguousarray(bias[_PERM])[None, :].astype(BF16)

    if _NC_CACHE is None:
        _NC_CACHE = _build_bass()
    nc = _NC_CACHE

    in_maps = []
    for ci in range(NCORES):
        txs = tx[ci * BL : (ci + 1) * BL]                     # [BL, T, D]
        txT = np.ascontiguousarray(
            txs.reshape(BL * T, D).T                           # [D, BL*T]
        ).astype(BF16)
        in_maps.append(
            {
                "txT": txT,
                "kern_perm": kern_perm,
                "r_perm": r_perm,
                "bias_perm": bias_perm,
            }
        )

    res = run_bass_kernel_spmd(nc, in_maps, core_ids=list(range(NCORES)))
    LAST_RESULTS = res
    h_last = np.empty((B, U), dtype=np.float32)
    for ci in range(NCORES):
        hT = res.results[ci]["hT_out"]                        # [JL, HB, BL] f32
        # h[b, 128*hb + jl] = hT[jl, hb, b]
        h_last[ci * BL : (ci + 1) * BL] = hT.transpose(2, 1, 0).reshape(BL, U)

    logits = h_last @ fc_w + fc_b
    e = np.exp(logits - logits.max(axis=1, keepdims=True))
    return (e / e.sum(axis=1, keepdims=True)).astype(np.float32)
